# revision 35
# baseline (speedup 1.0000x reference)
"""Trainium2 Bass kernel for nn_ExactModel_15092515078731.

Reference computes, per timestep t:
    U = expm(-i t H);  psi = U[:, 0]
    rotate psi by 32 per-observable tensor-product single-qubit bases
    probs = |rotated|^2 ; gather at indices

Algorithm: Krylov (Lanczos) projection.  H is real-symmetric, so
psi_t = expm(-itH) e0 ~= V exp(-itT) e1 with V the (t-independent!)
m=32-vector Lanczos basis of K(H, e0) and T the 32x32 tridiagonal
projection, both built on host (the original baseline already ran 80
host Lanczos iterations just for spectral bounds).  Per-t coefficient
vectors y_t = exp(-itT) e1 are tiny (32 complex).

Device work per core (SPMD over 8 cores, sharded by OBSERVABLE --
each core owns 4 of the 32 observables for all 8 timesteps):
  0. junk warm-up matmuls with no data deps keep the PE busy from block
     entry so its DVFS p-state ramps to full clock during the input-DMA
     wait (and through the rr-copy gap after the evolution).
  1. evolution: psi_{t,r}[p, q] = sum_k V[(q<<7)|p, k] y^r_t[k] for all
     8 t and r in {re, im, -im, re}: 8 fp16 matmuls of 128 cols via a
     block-diagonal y trick (4 q-values x 32 k on the contraction
     partitions), all into one PSUM tile.
  2. rr-copies (DVE): 4 strided PSUM->SBUF f16 copies produce cat1
     ([psi_re | psi_im] per t) and cat2 ([-psi_im | psi_re]) tiles --
     the (r, q)-major stationary operands for stage A.  Separate
     single-writer tiles: the tile framework serializes same-tile
     writers/interleaved readers across engines.
  3. rotation stage A (swapped operands: state stationary, Wpart
     moving -- no transposes anywhere): per t, 4 matmuls of 256 cols
     accumulate psA[(u, r, q), (bb, p')] = [T_re; T_im] for bl = 2bb+u,
     with the u-half placed via the out-AP partition base (PE quadrant
     placement).  Casts to fp16 sbA tiles keyed (h, bb), engine keyed
     by bb so DVE/ACT run concurrently per t.
  4. rotation stage B: one full-width c=128 matmul per (t-half, bb):
     block-diag-over-u complex-structured Wfree blocks [128, 128].
  5. |.|^2: ACT squares [128, 512] -> fp16; the re^2+im^2 partition
     pair-add and the index gather happen on host (the squared tiles
     DMA out directly, 4 chunks pipelined on the sync queue).
Host does only small parameter prep (Lanczos on one 4096-vector,
rotation kron products) and the final gather.
"""
import sys

if "/opt/trn_rl_repo" not in sys.path:
    sys.path.insert(0, "/opt/trn_rl_repo")

from contextlib import ExitStack

import numpy as np

import concourse.bacc as bacc
import concourse.bass as bass  # noqa: F401
import concourse.mybir as mybir
import concourse.tile as tile
from concourse.bass_utils import run_bass_kernel_spmd

N = 12
DIM = 4096
P = 128    # partition: bits 0-6
F = 32     # free: bits 7-11
NCORES = 8
B = 32     # observables
BPC = 4    # observables per core
T = 8      # timesteps
M = 32     # Krylov dimension

_s = 1.0 / np.sqrt(2.0)
U_BASIS = np.stack([
    np.array([[1, 1], [1, -1]]) * _s,
    np.array([[1, -1j], [1, 1j]]) * _s,
    np.eye(2),
]).astype(np.complex128)

F32 = mybir.dt.float32
F16 = mybir.dt.float16
MULT = mybir.AluOpType.mult
ADD = mybir.AluOpType.add


# ----------------------------------------------------------------------------
# host math
# ----------------------------------------------------------------------------

def _build_zz_diag(params_zz):
    basis = np.arange(DIM)
    bits = (basis[:, None] >> np.arange(N)[None, :]) & 1
    signs = (1 - 2 * bits).astype(np.float64)
    return (signs[:, :-1] * signs[:, 1:]) @ params_zz


def _h_matvec(v, params_x, zz_diag):
    out = zz_diag * v
    idx = np.arange(DIM)
    for i in range(N):
        out = out + params_x[i] * v[idx ^ (1 << i)]
    return out


def _lanczos(params_x, zz_diag, m=M):
    """m-step Lanczos of H from e0 with full reorthogonalization.
    Returns V (DIM, m).  On breakdown the remaining columns stay zero
    (the Krylov space is then invariant and the projection exact)."""
    V = np.zeros((DIM, m))
    V[0, 0] = 1.0
    for j in range(m - 1):
        w = _h_matvec(V[:, j], params_x, zz_diag)
        for _ in range(2):
            w = w - V[:, :j + 1] @ (V[:, :j + 1].T @ w)
        beta = np.linalg.norm(w)
        if beta < 1e-10:
            break
        V[:, j + 1] = w / beta
    return V


def _build_rot_mats(pauli_obs):
    """Wpart (B,128,128), Wfree (B,32,32); qubit acting on bit k is
    U_BASIS[pauli_obs[b, 11-k]] (reference reshape is bit-11-major)."""
    Wpart = np.zeros((B, P, P), np.complex128)
    Wfree = np.zeros((B, F, F), np.complex128)
    for b in range(B):
        Ub = [U_BASIS[pauli_obs[b, 11 - k]] for k in range(N)]
        wp = np.array([[1.0]])
        for k in range(6, -1, -1):
            wp = np.kron(wp, Ub[k])
        wf = np.array([[1.0]])
        for k in range(11, 6, -1):
            wf = np.kron(wf, Ub[k])
        Wpart[b] = wp
        Wfree[b] = wf
    return Wpart, Wfree


def prepare_host_data(initial_state, ts, pauli_obs, params_x, params_zz):
    """Returns (shared dict, per-core list of dicts)."""
    n0 = int(initial_state)
    assert n0 == 0
    ts = np.asarray(ts, np.float64)
    pauli_obs = np.asarray(pauli_obs, np.int64)
    params_x = np.asarray(params_x, np.float64)
    params_zz = np.asarray(params_zz, np.float64)

    zz_diag = _build_zz_diag(params_zz)
    V = _lanczos(params_x, zz_diag)                       # (DIM, M)
    HV = np.stack([_h_matvec(V[:, k], params_x, zz_diag)
                   for k in range(M)], axis=1)
    Tm = V.T @ HV                                          # (M, M)
    wT, QT = np.linalg.eigh(Tm)
    # y_t = exp(-i t T) e1
    ys = [QT @ (np.exp(-1j * t * wT) * QT[0, :]) for t in ts]

    # V in evolution lhsT layout: V16[(j,k), g*128+p] = V[((4g+j)<<7)|p, k]
    Vr = V.reshape(F, P, M)                                # [q, p, k]
    V16 = np.zeros((P, 8 * P), np.float16)
    for g in range(8):
        for j in range(4):
            # rows j*32+k, cols g*128+p
            V16[j * M:(j + 1) * M, g * P:(g + 1) * P] = \
                Vr[4 * g + j].T.astype(np.float16)
    # Y block-diag, cols (t-half, j', t-local, rr):
    # Y[(j,k), th*64 + j'*16 + tl*4 + rr] = (j==j') * y^rr_{4th+tl}[k]
    Y16 = np.zeros((P, P), np.float16)
    for t in range(T):
        th, tl = t // 4, t % 4
        yre = ys[t].real
        yim = ys[t].imag
        for j in range(4):
            rows = np.s_[j * M:(j + 1) * M]
            base = th * 64 + j * 16 + tl * 4
            Y16[rows, base + 0] = yre.astype(np.float16)
            Y16[rows, base + 1] = yim.astype(np.float16)
            Y16[rows, base + 2] = (-yim).astype(np.float16)
            Y16[rows, base + 3] = yre.astype(np.float16)
    VY = np.concatenate([Y16, V16], axis=1)                # (128, 1152)

    Wpart, Wfree = _build_rot_mats(pauli_obs)
    per_core = []
    for c in range(NCORES):
        bs = [BPC * c + i for i in range(BPC)]
        # stage A moving weights: WA[p, w*512 + u*256 + bb*128 + p'] =
        # re/im W[p',p] for bl = 2*bb + u  (u selects the psA partition half)
        WA = np.zeros((P, 2 * BPC * P), np.float16)
        for bl, b in enumerate(bs):
            u, bb = bl % 2, bl // 2
            WA[:, u * 2 * P + bb * P:u * 2 * P + (bb + 1) * P] = \
                Wpart[b].T.real.astype(np.float16)
            WA[:, BPC * P + u * 2 * P + bb * P:
                BPC * P + u * 2 * P + (bb + 1) * P] = \
                Wpart[b].T.imag.astype(np.float16)
        # stage B stationary: per bb a [128, 128] block-diag over u of the
        # complex-structured Wfree blocks
        WF = np.zeros((4 * F, 2 * 4 * F), np.float16)
        for bl, b in enumerate(bs):
            u, bb = bl % 2, bl // 2
            fre = Wfree[b].real.T.astype(np.float16)   # [q, q'] = Wf[q',q]
            fim = Wfree[b].imag.T.astype(np.float16)
            r0 = 2 * F * u
            c0 = 4 * F * bb + 2 * F * u
            WF[r0:r0 + F, c0:c0 + F] = fre
            WF[r0:r0 + F, c0 + F:c0 + 2 * F] = fim
            WF[r0 + F:r0 + 2 * F, c0:c0 + F] = -fim
            WF[r0 + F:r0 + 2 * F, c0 + F:c0 + 2 * F] = fre
        per_core.append(dict(wa=WA, wf=WF))
    shared = dict(vy=VY)
    return shared, per_core


# ----------------------------------------------------------------------------
# device program
# ----------------------------------------------------------------------------

def _copy(eng, out, in_):
    if hasattr(eng, "tensor_copy"):
        eng.tensor_copy(out, in_)
    else:
        eng.copy(out, in_)


def build_program():
    nc = bacc.Bacc("TRN2", target_bir_lowering=False, debug=False,
                   num_devices=NCORES)

    d_vy = nc.dram_tensor("vy", [P, 9 * P], F16, kind="ExternalInput")
    d_wa = nc.dram_tensor("wa", [P, 2 * BPC * P], F16, kind="ExternalInput")
    d_wf = nc.dram_tensor("wf", [4 * F, 2 * 4 * F], F16,
                          kind="ExternalInput")
    d_sq = nc.dram_tensor("sqout", [P, 4 * BPC * P], F16,
                          kind="ExternalOutput")

    with tile.TileContext(nc) as tc, ExitStack() as ctx:
        consts = ctx.enter_context(tc.tile_pool(name="consts", bufs=1))
        work = ctx.enter_context(tc.tile_pool(name="work", bufs=1))
        sq_pool = ctx.enter_context(tc.tile_pool(name="sq", bufs=4))
        ps_psi = ctx.enter_context(tc.tile_pool(name="ps_psi", bufs=1,
                                                space="PSUM"))
        ps_a = ctx.enter_context(tc.tile_pool(name="ps_a", bufs=4,
                                              space="PSUM"))
        ps_b = ctx.enter_context(tc.tile_pool(name="ps_b", bufs=2,
                                              space="PSUM"))

        # dummy ACT op to trigger the activation-table load during the
        # input-DMA wait instead of at the first real ACT use
        sb_dummy = consts.tile([P, 8], F32, tag="dummy")
        nc.gpsimd.memset(sb_dummy, 0.0)
        nc.scalar.square(sb_dummy, sb_dummy)

        # input DMAs: [Y | V cols 0-383] on sync, rest of V on scalar, then
        # the rotation weights
        sb_vy = consts.tile([P, 9 * P], F16, tag="vy")
        nc.sync.dma_start(out=sb_vy[:, 0:5 * P], in_=d_vy.ap()[:, 0:5 * P])
        nc.scalar.dma_start(out=sb_vy[:, 5 * P:9 * P],
                            in_=d_vy.ap()[:, 5 * P:9 * P])
        sb_wa = consts.tile([P, 2 * BPC * P], F16, tag="wa")
        nc.sync.dma_start(out=sb_wa, in_=d_wa.ap())
        sb_wf = consts.tile([4 * F, 2 * 4 * F], F16, tag="wf")
        nc.scalar.dma_start(out=sb_wf, in_=d_wf.ap())

        # PE warm-up: junk matmuls with no data deps keep the tensor engine
        # busy through the input-DMA wait so its p-state ramps to full clock
        # before the real work arrives
        sb_junk = consts.tile([P, 4 * P], F16, tag="junk")
        nc.gpsimd.memset(sb_junk, 0.0)
        ps_junk = ps_b.tile([P, 4 * P], F32, tag="psB")
        for _ in range(3):
            nc.tensor.matmul(ps_junk, sb_junk[:, 0:P], sb_junk,
                             start=True, stop=True, skip_group_check=True)

        # ---------------- evolution: psi for all 8 t ----------------
        # split by t-half: psi_th[p, (g, j, tl, rr)] via 8 matmuls of 64
        # cols each; the rr-copies for half 0 run on DVE while the PE is
        # still doing half 1, so stage A can start ~1us earlier
        psi_0 = ps_psi.tile([P, 4 * P], F32, tag="psi0")
        psi_1 = ps_psi.tile([P, 4 * P], F32, tag="psi1")
        psi_h = [psi_0, psi_1]
        cat1_0 = work.tile([P, 4 * 2 * F], F16, tag="cat10")
        cat1_1 = work.tile([P, 4 * 2 * F], F16, tag="cat11")
        cat2_0 = work.tile([P, 4 * 2 * F], F16, tag="cat20")
        cat2_1 = work.tile([P, 4 * 2 * F], F16, tag="cat21")
        cat1_h = [cat1_0, cat1_1]
        cat2_h = [cat2_0, cat2_1]

        def rr_copies(th):
            ps_r = psi_h[th].rearrange("p (gj tl rr) -> p rr tl gj",
                                       gj=F, tl=4, rr=4)
            c1_r = cat1_h[th].rearrange("p (tl rr gj) -> p rr tl gj",
                                        gj=F, tl=4, rr=2)
            c2_r = cat2_h[th].rearrange("p (tl rr gj) -> p rr tl gj",
                                        gj=F, tl=4, rr=2)
            _copy(nc.vector, c1_r[:, 0], ps_r[:, 0])
            _copy(nc.vector, c1_r[:, 1], ps_r[:, 1])
            _copy(nc.vector, c2_r[:, 0], ps_r[:, 2])
            _copy(nc.vector, c2_r[:, 1], ps_r[:, 3])

        for th in range(2):
            for g in range(8):
                nc.tensor.matmul(psi_h[th][:, g * 2 * F:(g + 1) * 2 * F],
                                 sb_vy[:, (1 + g) * P:(2 + g) * P],
                                 sb_vy[:, th * 2 * F:(th + 1) * 2 * F],
                                 start=True, stop=True,
                                 skip_group_check=True)
            rr_copies(th)

        # single bridge junk matmul: cover the short PE gap between the
        # last evolution matmul and stage A so the DVFS ramp isn't reset
        nc.tensor.matmul(ps_junk, sb_junk[:, 0:P], sb_junk,
                         start=True, stop=True, skip_group_check=True)

        # ---------------- rotation stage A ----------------
        # psA_t[(r q), (bl p')] = [T_re; T_im]; casts split per observable
        # pair into per-(h, bb) tiles, engine keyed by bb (single writer per
        # tile, both engines run concurrently per t)
        sb_a00 = work.tile([4 * F, 4 * P], F16, tag="sba00")
        sb_a01 = work.tile([4 * F, 4 * P], F16, tag="sba01")
        sb_a10 = work.tile([4 * F, 4 * P], F16, tag="sba10")
        sb_a11 = work.tile([4 * F, 4 * P], F16, tag="sba11")
        sb_ahb = [[sb_a00, sb_a01], [sb_a10, sb_a11]]
        aeng = [nc.vector, nc.scalar]

        def stage_a(t):
            # psA[(u, r, q), (bb, p')]: u-halves placed by the PE via the
            # out-AP partition base; 4 matmuls of 256 cols accumulate re/im
            psA = ps_a.tile([4 * F, 2 * P], F32, tag="psA")
            c1 = cat1_h[t // 4][:, (t % 4) * 2 * F:(t % 4 + 1) * 2 * F]
            c2 = cat2_h[t // 4][:, (t % 4) * 2 * F:(t % 4 + 1) * 2 * F]
            for u in range(2):
                sl = np.s_[2 * F * u:2 * F * (u + 1), :]
                nc.tensor.matmul(psA[sl], c1,
                                 sb_wa[:, u * 2 * P:(u + 1) * 2 * P],
                                 start=True, stop=False,
                                 skip_group_check=True)
                nc.tensor.matmul(psA[sl], c2,
                                 sb_wa[:, (2 + u) * 2 * P:(3 + u) * 2 * P],
                                 start=False, stop=True,
                                 skip_group_check=True)
            for bb in range(2):
                dst = sb_ahb[t // 4][bb][:, (t % 4) * P:(t % 4 + 1) * P]
                _copy(aeng[bb], dst, psA[:, bb * P:(bb + 1) * P])

        # ---------------- stage B + |.|^2 + out ----------------
        # psB packs two observables on the partition axis (PE quadrant
        # placement); squares on ACT; re^2+im^2 pair-add happens on host
        def stage_b(h, bb, split=False):
            psB = ps_b.tile([P, BPC * P], F32, tag="psB")
            nc.tensor.matmul(psB, sb_wf[:, bb * P:(bb + 1) * P],
                             sb_ahb[h][bb], start=True, stop=True)
            sq = sq_pool.tile([P, BPC * P], F16, tag="sq")
            c0 = (2 * h + bb) * BPC * P
            if split:
                # final chunk: half-width squares + DMAs pipeline the tail
                for s in range(2):
                    sl = np.s_[:, s * 2 * P:(s + 1) * 2 * P]
                    nc.scalar.square(sq[sl], psB[sl])
                    nc.sync.dma_start(
                        out=d_sq.ap()[:, c0 + s * 2 * P:c0 + (s + 1) * 2 * P],
                        in_=sq[sl])
            else:
                nc.scalar.square(sq, psB)
                nc.sync.dma_start(out=d_sq.ap()[:, c0:c0 + BPC * P], in_=sq)

        for t in range(4):
            stage_a(t)
        stage_a(4)
        stage_b(0, 0)
        stage_a(5)
        stage_a(6)
        stage_b(0, 1)
        stage_a(7)
        stage_b(1, 0)
        stage_b(1, 1, split=True)

    nc.compile()
    return nc


# ----------------------------------------------------------------------------
# entry point
# ----------------------------------------------------------------------------

_PROGRAM_CACHE = {}

# test-harness knobs (grading path leaves these untouched)
TRACE = False
LAST_RESULT = None


def kernel(initial_state, ts, pauli_obs, indices, params_x, params_zz):
    ts = np.asarray(ts)
    pauli_obs = np.asarray(pauli_obs)
    indices = np.asarray(indices)
    Tn = ts.shape[0]
    shots = indices.shape[2]
    assert Tn == T, f"expected {T} timesteps, got {Tn}"

    shared, per_core = prepare_host_data(
        initial_state, ts, pauli_obs, params_x, params_zz)

    if "prog" not in _PROGRAM_CACHE:
        _PROGRAM_CACHE["prog"] = build_program()
    nc = _PROGRAM_CACHE["prog"]

    in_maps = [{**shared, **pc} for pc in per_core]
    res = run_bass_kernel_spmd(nc, in_maps, core_ids=list(range(NCORES)),
                               trace=TRACE)
    global LAST_RESULT
    LAST_RESULT = res

    out = np.zeros((Tn, B, shots), np.float32)
    idx = indices.astype(np.int64)
    for c in range(NCORES):
        tiles = np.asarray(res.results[c]["sqout"], np.float32)  # (128, 2048)
        # chunk (h, bb) at cols (2h+bb)*512; rows 64u+32r+q'; cols (t%4, p')
        ch = tiles.reshape(P, 2, 2, 4, P).transpose(1, 2, 0, 3, 4)
        ch = ch.reshape(2, 2, 2, 2, F, 4, P)        # [h, bb, u, r, q', t4, p']
        pr = ch.sum(axis=3)                          # re^2 + im^2
        # -> [t, bl, n]: t = 4h + t4, bl = 2bb + u, n = q'<<7 | p'
        pr = pr.transpose(0, 4, 1, 2, 3, 5).reshape(Tn, BPC, DIM)
        for bl in range(BPC):
            b = BPC * c + bl
            out[:, b, :] = np.take_along_axis(pr[:, bl], idx[:, b], axis=1)
    return out


# revision 36
# speedup vs baseline: 1.1818x; 1.1818x over previous
"""Trainium2 Bass kernel for nn_ExactModel_15092515078731.

Reference computes, per timestep t:
    U = expm(-i t H);  psi = U[:, 0]
    rotate psi by 32 per-observable tensor-product single-qubit bases
    probs = |rotated|^2 ; gather at indices

Algorithm: Krylov (Lanczos) projection.  H is real-symmetric, so
psi_t = expm(-itH) e0 ~= V exp(-itT) e1 with V the (t-independent!)
m=32-vector Lanczos basis of K(H, e0) and T the 32x32 tridiagonal
projection, both built on host (the original baseline already ran 80
host Lanczos iterations just for spectral bounds).  Per-t coefficient
vectors y_t = exp(-itT) e1 are tiny (32 complex).

Device work per core (SPMD over 8 cores, sharded by OBSERVABLE --
each core owns 4 of the 32 observables for all 8 timesteps):
  0. junk warm-up matmuls with no data deps keep the PE busy from block
     entry so its DVFS p-state ramps to full clock during the input-DMA
     wait (and through the rr-copy gap after the evolution).
  1. evolution: psi_{t,r}[p, q] = sum_k V[(q<<7)|p, k] y^r_t[k] for all
     8 t and r in {re, im, -im, re}: 8 fp16 matmuls of 128 cols via a
     block-diagonal y trick (4 q-values x 32 k on the contraction
     partitions), all into one PSUM tile.
  2. rr-copies (DVE): 4 strided PSUM->SBUF f16 copies produce cat1
     ([psi_re | psi_im] per t) and cat2 ([-psi_im | psi_re]) tiles --
     the (r, q)-major stationary operands for stage A.  Separate
     single-writer tiles: the tile framework serializes same-tile
     writers/interleaved readers across engines.
  3. rotation stage A (swapped operands: state stationary, Wpart
     moving -- no transposes anywhere): per t, 4 matmuls of 256 cols
     accumulate psA[(u, r, q), (bb, p')] = [T_re; T_im] for bl = 2bb+u,
     with the u-half placed via the out-AP partition base (PE quadrant
     placement).  Casts to fp16 sbA tiles keyed (h, bb), engine keyed
     by bb so DVE/ACT run concurrently per t.
  4. rotation stage B: one full-width c=128 matmul per (t-half, bb):
     block-diag-over-u complex-structured Wfree blocks [128, 128].
  5. |.|^2: ACT squares [128, 512] -> fp16; the re^2+im^2 partition
     pair-add and the index gather happen on host (the squared tiles
     DMA out directly, 4 chunks pipelined on the sync queue).
Host does only small parameter prep (Lanczos on one 4096-vector,
rotation kron products) and the final gather.
"""
import sys

if "/opt/trn_rl_repo" not in sys.path:
    sys.path.insert(0, "/opt/trn_rl_repo")

from contextlib import ExitStack

import numpy as np

import concourse.bacc as bacc
import concourse.bass as bass  # noqa: F401
import concourse.mybir as mybir
import concourse.tile as tile
from concourse.bass_utils import run_bass_kernel_spmd

N = 12
DIM = 4096
P = 128    # partition: bits 0-6
F = 32     # free: bits 7-11
NCORES = 8
B = 32     # observables
BPC = 4    # observables per core
T = 8      # timesteps
M = 32     # Krylov dimension

_s = 1.0 / np.sqrt(2.0)
U_BASIS = np.stack([
    np.array([[1, 1], [1, -1]]) * _s,
    np.array([[1, -1j], [1, 1j]]) * _s,
    np.eye(2),
]).astype(np.complex128)

F32 = mybir.dt.float32
F16 = mybir.dt.float16
MULT = mybir.AluOpType.mult
ADD = mybir.AluOpType.add


# ----------------------------------------------------------------------------
# host math
# ----------------------------------------------------------------------------

def _build_zz_diag(params_zz):
    basis = np.arange(DIM)
    bits = (basis[:, None] >> np.arange(N)[None, :]) & 1
    signs = (1 - 2 * bits).astype(np.float64)
    return (signs[:, :-1] * signs[:, 1:]) @ params_zz


def _h_matvec(v, params_x, zz_diag):
    out = zz_diag * v
    idx = np.arange(DIM)
    for i in range(N):
        out = out + params_x[i] * v[idx ^ (1 << i)]
    return out


def _lanczos(params_x, zz_diag, m=M):
    """m-step Lanczos of H from e0 with full reorthogonalization.
    Returns V (DIM, m).  On breakdown the remaining columns stay zero
    (the Krylov space is then invariant and the projection exact)."""
    V = np.zeros((DIM, m))
    V[0, 0] = 1.0
    for j in range(m - 1):
        w = _h_matvec(V[:, j], params_x, zz_diag)
        for _ in range(2):
            w = w - V[:, :j + 1] @ (V[:, :j + 1].T @ w)
        beta = np.linalg.norm(w)
        if beta < 1e-10:
            break
        V[:, j + 1] = w / beta
    return V


def _build_rot_mats(pauli_obs):
    """Wpart (B,128,128), Wfree (B,32,32); qubit acting on bit k is
    U_BASIS[pauli_obs[b, 11-k]] (reference reshape is bit-11-major)."""
    Wpart = np.zeros((B, P, P), np.complex128)
    Wfree = np.zeros((B, F, F), np.complex128)
    for b in range(B):
        Ub = [U_BASIS[pauli_obs[b, 11 - k]] for k in range(N)]
        wp = np.array([[1.0]])
        for k in range(6, -1, -1):
            wp = np.kron(wp, Ub[k])
        wf = np.array([[1.0]])
        for k in range(11, 6, -1):
            wf = np.kron(wf, Ub[k])
        Wpart[b] = wp
        Wfree[b] = wf
    return Wpart, Wfree


def prepare_host_data(initial_state, ts, pauli_obs, params_x, params_zz):
    """Returns (shared dict, per-core list of dicts)."""
    n0 = int(initial_state)
    assert n0 == 0
    ts = np.asarray(ts, np.float64)
    pauli_obs = np.asarray(pauli_obs, np.int64)
    params_x = np.asarray(params_x, np.float64)
    params_zz = np.asarray(params_zz, np.float64)

    zz_diag = _build_zz_diag(params_zz)
    V = _lanczos(params_x, zz_diag)                       # (DIM, M)
    HV = np.stack([_h_matvec(V[:, k], params_x, zz_diag)
                   for k in range(M)], axis=1)
    Tm = V.T @ HV                                          # (M, M)
    wT, QT = np.linalg.eigh(Tm)
    # y_t = exp(-i t T) e1
    ys = [QT @ (np.exp(-1j * t * wT) * QT[0, :]) for t in ts]

    # V in evolution lhsT layout: V16[(j,k), g*128+p] = V[((4g+j)<<7)|p, k]
    Vr = V.reshape(F, P, M)                                # [q, p, k]
    V16 = np.zeros((P, 8 * P), np.float16)
    for g in range(8):
        for j in range(4):
            # rows j*32+k, cols g*128+p
            V16[j * M:(j + 1) * M, g * P:(g + 1) * P] = \
                Vr[4 * g + j].T.astype(np.float16)
    # Y block-diag, cols (t-half, j', t-local, rr):
    # Y[(j,k), th*64 + j'*16 + tl*4 + rr] = (j==j') * y^rr_{4th+tl}[k]
    Y16 = np.zeros((P, P), np.float16)
    for t in range(T):
        th, tl = t // 4, t % 4
        yre = ys[t].real
        yim = ys[t].imag
        for j in range(4):
            rows = np.s_[j * M:(j + 1) * M]
            base = th * 64 + j * 16 + tl * 4
            Y16[rows, base + 0] = yre.astype(np.float16)
            Y16[rows, base + 1] = yim.astype(np.float16)
            Y16[rows, base + 2] = (-yim).astype(np.float16)
            Y16[rows, base + 3] = yre.astype(np.float16)
    VY = np.concatenate([Y16, V16], axis=1)                # (128, 1152)

    Wpart, Wfree = _build_rot_mats(pauli_obs)
    per_core = []
    for c in range(NCORES):
        bs = [BPC * c + i for i in range(BPC)]
        # stage A moving weights: WA[p, w*512 + u*256 + bb*128 + p'] =
        # re/im W[p',p] for bl = 2*bb + u  (u selects the psA partition half)
        WA = np.zeros((P, 2 * BPC * P), np.float16)
        for bl, b in enumerate(bs):
            u, bb = bl % 2, bl // 2
            WA[:, u * 2 * P + bb * P:u * 2 * P + (bb + 1) * P] = \
                Wpart[b].T.real.astype(np.float16)
            WA[:, BPC * P + u * 2 * P + bb * P:
                BPC * P + u * 2 * P + (bb + 1) * P] = \
                Wpart[b].T.imag.astype(np.float16)
        # stage B stationary: per bb a [128, 128] block-diag over u of the
        # complex-structured Wfree blocks
        WF = np.zeros((4 * F, 2 * 4 * F), np.float16)
        for bl, b in enumerate(bs):
            u, bb = bl % 2, bl // 2
            fre = Wfree[b].real.T.astype(np.float16)   # [q, q'] = Wf[q',q]
            fim = Wfree[b].imag.T.astype(np.float16)
            r0 = 2 * F * u
            c0 = 4 * F * bb + 2 * F * u
            WF[r0:r0 + F, c0:c0 + F] = fre
            WF[r0:r0 + F, c0 + F:c0 + 2 * F] = fim
            WF[r0 + F:r0 + 2 * F, c0:c0 + F] = -fim
            WF[r0 + F:r0 + 2 * F, c0 + F:c0 + 2 * F] = fre
        per_core.append(dict(wa=WA, wf=WF))
    shared = dict(vy=VY)
    return shared, per_core


# ----------------------------------------------------------------------------
# device program
# ----------------------------------------------------------------------------

def _copy(eng, out, in_):
    if hasattr(eng, "tensor_copy"):
        eng.tensor_copy(out, in_)
    else:
        eng.copy(out, in_)


def build_program():
    nc = bacc.Bacc("TRN2", target_bir_lowering=False, debug=False,
                   num_devices=NCORES)

    d_vy = nc.dram_tensor("vy", [P, 9 * P], F16, kind="ExternalInput")
    d_wa = nc.dram_tensor("wa", [P, 2 * BPC * P], F16, kind="ExternalInput")
    d_wf = nc.dram_tensor("wf", [4 * F, 2 * 4 * F], F16,
                          kind="ExternalInput")
    d_sq = nc.dram_tensor("sqout", [P, 4 * BPC * P], F16,
                          kind="ExternalOutput")

    with tile.TileContext(nc) as tc, ExitStack() as ctx:
        consts = ctx.enter_context(tc.tile_pool(name="consts", bufs=1))
        work = ctx.enter_context(tc.tile_pool(name="work", bufs=1))
        sq_pool = ctx.enter_context(tc.tile_pool(name="sq", bufs=4))
        ps_psi = ctx.enter_context(tc.tile_pool(name="ps_psi", bufs=1,
                                                space="PSUM"))
        ps_a = ctx.enter_context(tc.tile_pool(name="ps_a", bufs=4,
                                              space="PSUM"))
        ps_b = ctx.enter_context(tc.tile_pool(name="ps_b", bufs=2,
                                              space="PSUM"))

        # dummy ACT op to trigger the activation-table load during the
        # input-DMA wait instead of at the first real ACT use
        sb_dummy = consts.tile([P, 8], F32, tag="dummy")
        nc.gpsimd.memset(sb_dummy, 0.0)
        nc.scalar.square(sb_dummy, sb_dummy)

        # input DMAs: [Y | V cols 0-383] on sync, rest of V on scalar, then
        # the rotation weights
        sb_vy = consts.tile([P, 9 * P], F16, tag="vy")
        nc.sync.dma_start(out=sb_vy[:, 0:5 * P], in_=d_vy.ap()[:, 0:5 * P])
        nc.scalar.dma_start(out=sb_vy[:, 5 * P:9 * P],
                            in_=d_vy.ap()[:, 5 * P:9 * P])
        sb_wa = consts.tile([P, 2 * BPC * P], F16, tag="wa")
        nc.sync.dma_start(out=sb_wa, in_=d_wa.ap())
        sb_wf = consts.tile([4 * F, 2 * 4 * F], F16, tag="wf")
        nc.scalar.dma_start(out=sb_wf, in_=d_wf.ap())

        # PE warm-up: junk matmuls with no data deps keep the tensor engine
        # busy through the input-DMA wait so its p-state ramps to full clock
        # before the real work arrives
        sb_junk = consts.tile([P, 4 * P], F16, tag="junk")
        nc.gpsimd.memset(sb_junk, 0.0)
        ps_junk = ps_b.tile([P, 4 * P], F32, tag="psB")
        for _ in range(3):
            nc.tensor.matmul(ps_junk, sb_junk[:, 0:P], sb_junk,
                             start=True, stop=True, skip_group_check=True)

        # ---------------- evolution: psi for all 8 t ----------------
        # split by t-half: psi_th[p, (g, j, tl, rr)] via 8 matmuls of 64
        # cols each; the rr-copies for half 0 run on DVE while the PE is
        # still doing half 1, so stage A can start ~1us earlier
        psi_0 = ps_psi.tile([P, 4 * P], F32, tag="psi0")
        psi_1 = ps_psi.tile([P, 4 * P], F32, tag="psi1")
        psi_h = [psi_0, psi_1]
        cat1_0 = work.tile([P, 4 * 2 * F], F16, tag="cat10")
        cat1_1 = work.tile([P, 4 * 2 * F], F16, tag="cat11")
        cat2_0 = work.tile([P, 4 * 2 * F], F16, tag="cat20")
        cat2_1 = work.tile([P, 4 * 2 * F], F16, tag="cat21")
        cat1_h = [cat1_0, cat1_1]
        cat2_h = [cat2_0, cat2_1]

        def rr_copies(th):
            ps_r = psi_h[th].rearrange("p (gj tl rr) -> p rr tl gj",
                                       gj=F, tl=4, rr=4)
            c1_r = cat1_h[th].rearrange("p (tl rr gj) -> p rr tl gj",
                                        gj=F, tl=4, rr=2)
            c2_r = cat2_h[th].rearrange("p (tl rr gj) -> p rr tl gj",
                                        gj=F, tl=4, rr=2)
            _copy(nc.vector, c1_r[:, 0], ps_r[:, 0])
            _copy(nc.vector, c1_r[:, 1], ps_r[:, 1])
            _copy(nc.vector, c2_r[:, 0], ps_r[:, 2])
            _copy(nc.vector, c2_r[:, 1], ps_r[:, 3])

        for th in range(2):
            for g in range(8):
                nc.tensor.matmul(psi_h[th][:, g * 2 * F:(g + 1) * 2 * F],
                                 sb_vy[:, (1 + g) * P:(2 + g) * P],
                                 sb_vy[:, th * 2 * F:(th + 1) * 2 * F],
                                 start=True, stop=True,
                                 skip_group_check=True)
            rr_copies(th)

        # single bridge junk matmul: cover the short PE gap between the
        # last evolution matmul and stage A so the DVFS ramp isn't reset
        nc.tensor.matmul(ps_junk, sb_junk[:, 0:P], sb_junk,
                         start=True, stop=True, skip_group_check=True)

        # ---------------- rotation stage A ----------------
        # psA_t[(r q), (bl p')] = [T_re; T_im]; casts split per observable
        # pair into per-(h, bb) tiles, engine keyed by bb (single writer per
        # tile, both engines run concurrently per t)
        sb_a00 = work.tile([4 * F, 4 * P], F16, tag="sba00")
        sb_a01 = work.tile([4 * F, 4 * P], F16, tag="sba01")
        sb_a10 = work.tile([4 * F, 4 * P], F16, tag="sba10")
        sb_a11 = work.tile([4 * F, 4 * P], F16, tag="sba11")
        sb_ahb = [[sb_a00, sb_a01], [sb_a10, sb_a11]]
        aeng = [nc.vector, nc.scalar]

        def stage_a(t):
            # psA[(u, r, q), (bb, p')]: u-halves placed by the PE via the
            # out-AP partition base; 4 matmuls of 256 cols accumulate re/im
            psA = ps_a.tile([4 * F, 2 * P], F32, tag="psA")
            c1 = cat1_h[t // 4][:, (t % 4) * 2 * F:(t % 4 + 1) * 2 * F]
            c2 = cat2_h[t // 4][:, (t % 4) * 2 * F:(t % 4 + 1) * 2 * F]
            for u in range(2):
                sl = np.s_[2 * F * u:2 * F * (u + 1), :]
                nc.tensor.matmul(psA[sl], c1,
                                 sb_wa[:, u * 2 * P:(u + 1) * 2 * P],
                                 start=True, stop=False,
                                 skip_group_check=True)
                nc.tensor.matmul(psA[sl], c2,
                                 sb_wa[:, (2 + u) * 2 * P:(3 + u) * 2 * P],
                                 start=False, stop=True,
                                 skip_group_check=True)
            for bb in range(2):
                dst = sb_ahb[t // 4][bb][:, (t % 4) * P:(t % 4 + 1) * P]
                _copy(aeng[bb], dst, psA[:, bb * P:(bb + 1) * P])

        # ---------------- stage B + |.|^2 + out ----------------
        # psB packs two observables on the partition axis (PE quadrant
        # placement); squares on ACT; re^2+im^2 pair-add happens on host
        def stage_b(h, bb, split=False):
            psB = ps_b.tile([P, BPC * P], F32, tag="psB")
            nc.tensor.matmul(psB, sb_wf[:, bb * P:(bb + 1) * P],
                             sb_ahb[h][bb], start=True, stop=True)
            sq = sq_pool.tile([P, BPC * P], F16, tag="sq")
            c0 = (2 * h + bb) * BPC * P
            if split:
                # final chunk: half-width squares + DMAs pipeline the tail
                for s in range(2):
                    sl = np.s_[:, s * 2 * P:(s + 1) * 2 * P]
                    nc.scalar.square(sq[sl], psB[sl])
                    nc.sync.dma_start(
                        out=d_sq.ap()[:, c0 + s * 2 * P:c0 + (s + 1) * 2 * P],
                        in_=sq[sl])
            else:
                nc.scalar.square(sq, psB)
                nc.sync.dma_start(out=d_sq.ap()[:, c0:c0 + BPC * P], in_=sq)

        for t in range(4):
            stage_a(t)
        stage_a(4)
        stage_b(0, 0)
        stage_a(5)
        stage_a(6)
        stage_b(0, 1)
        stage_a(7)
        stage_b(1, 0)
        stage_b(1, 1)

    nc.compile()
    return nc


# ----------------------------------------------------------------------------
# entry point
# ----------------------------------------------------------------------------

_PROGRAM_CACHE = {}

# test-harness knobs (grading path leaves these untouched)
TRACE = False
LAST_RESULT = None


def kernel(initial_state, ts, pauli_obs, indices, params_x, params_zz):
    ts = np.asarray(ts)
    pauli_obs = np.asarray(pauli_obs)
    indices = np.asarray(indices)
    Tn = ts.shape[0]
    shots = indices.shape[2]
    assert Tn == T, f"expected {T} timesteps, got {Tn}"

    shared, per_core = prepare_host_data(
        initial_state, ts, pauli_obs, params_x, params_zz)

    if "prog" not in _PROGRAM_CACHE:
        _PROGRAM_CACHE["prog"] = build_program()
    nc = _PROGRAM_CACHE["prog"]

    in_maps = [{**shared, **pc} for pc in per_core]
    res = run_bass_kernel_spmd(nc, in_maps, core_ids=list(range(NCORES)),
                               trace=TRACE)
    global LAST_RESULT
    LAST_RESULT = res

    out = np.zeros((Tn, B, shots), np.float32)
    idx = indices.astype(np.int64)
    for c in range(NCORES):
        tiles = np.asarray(res.results[c]["sqout"], np.float32)  # (128, 2048)
        # chunk (h, bb) at cols (2h+bb)*512; rows 64u+32r+q'; cols (t%4, p')
        ch = tiles.reshape(P, 2, 2, 4, P).transpose(1, 2, 0, 3, 4)
        ch = ch.reshape(2, 2, 2, 2, F, 4, P)        # [h, bb, u, r, q', t4, p']
        pr = ch.sum(axis=3)                          # re^2 + im^2
        # -> [t, bl, n]: t = 4h + t4, bl = 2bb + u, n = q'<<7 | p'
        pr = pr.transpose(0, 4, 1, 2, 3, 5).reshape(Tn, BPC, DIM)
        for bl in range(BPC):
            b = BPC * c + bl
            out[:, b, :] = np.take_along_axis(pr[:, bl], idx[:, b], axis=1)
    return out
